# revision 28
# baseline (speedup 1.0000x reference)
"""3-layer GCN forward (GCNConv x3 + log_softmax) on 8 Trainium2 cores.

Strategy (self-contained; shapes hardcoded for N=100000, Cin=Ch=128,
Cout=47, 8 cores): A_hat = D^-1/2 (A+I) D^-1/2 fixed across layers, so
per layer out = dinv_dst * segsum_dst(dinv_src * (H @ W)) + b.

Host: permute nodes into 8 contiguous core blocks (degree-ranked
round-robin so all cores share one loop structure / NEFF). Per core,
edges are sorted into (dst-group g, source-quarter q) segments; each
segment is padded to 128-position tiles. The message gather uses
batched dma_gather instructions (int16 indices rebased per source
quarter of 25088 rows, 4 SWDGE queues in parallel), fetching bf16
feature rows of 256 B from the AllGathered Z replica.

Aggregation per tile of 128 messages: build a selection matrix
SEL[p, r] = (dst_id[p] == r) with one DVE is_equal op (4 tiles per op
via a stride-0 broadcast), then matmul(lhsT=SEL, rhs=messages) into
the group's PSUM accumulator. Bias is a rank-1 matmul
(binv x b, binv = 1/dinv so the later dinv_dst scale restores b).
Post per group: one fused scale+relu, PE transpose, next-layer GEMM,
dinv_src scale, zs write. AllGather (bf16) exchanges Z between layers.
Final layer: scale + log_softmax.
"""
import numpy as np
import ml_dtypes

NCORES = 8
N = 100000
NBLK = 12500
NPAD = 12544            # 98 * 128
NGRP = NPAD // 128      # 98
C = 128
COUT = 47
QROWS = 25088           # rows per source quarter (2 core blocks)
NQ = 4
GBLK = 4                # groups per gather-chunk block


def _preprocess(x, edge_index, W1, b1, W2, b2, W3, b3):
    x = np.asarray(x, np.float32)
    ei = np.asarray(edge_index)
    loop = np.arange(N, dtype=np.int64)
    src = np.concatenate([ei[0], loop]).astype(np.int64)
    dst = np.concatenate([ei[1], loop]).astype(np.int64)

    deg = np.bincount(dst, minlength=N).astype(np.float32)
    dinv = 1.0 / np.sqrt(np.maximum(deg, 1.0))

    rank = np.argsort(-deg, kind="stable")
    perm = np.empty(N, np.int64)
    for k in range(NCORES):
        perm[k * NBLK:(k + 1) * NBLK] = rank[k::NCORES]
    inv = np.empty(N, np.int64)
    inv[perm] = np.arange(N)

    srcp = inv[src]
    dstp = inv[dst]
    ksrc = srcp // NBLK
    srcg = ksrc * NPAD + (srcp - ksrc * NBLK)     # padded-global coords
    dinv_p = dinv[perm]

    ecore = dstp // NBLK
    rloc = dstp - ecore * NBLK
    grp = rloc // 128
    gpart = rloc % 128
    qq = srcg // QROWS

    # counts per (core, group, quarter) -> shared tile structure
    key = (ecore * NGRP + grp) * NQ + qq
    cnt = np.bincount(key, minlength=NCORES * NGRP * NQ) \
        .reshape(NCORES, NGRP, NQ)
    tiles_gq = np.maximum(
        np.ceil(cnt / 128).astype(np.int64).max(axis=0), 1)   # [NGRP, NQ]

    # quarter-major stream layout: for q: for g: tiles_gq[g, q] tiles
    tile_base_q = np.zeros(NQ + 1, np.int64)
    tile_base_q[1:] = np.cumsum(tiles_gq.sum(axis=0))
    # tile index of (g, q): tile_base_q[q] + cumsum over g
    tile_off_gq = np.zeros((NGRP, NQ), np.int64)
    for q in range(NQ):
        tile_off_gq[1:, q] = np.cumsum(tiles_gq[:-1, q])
    TT = int(tile_base_q[-1])          # total tiles per core per layer

    # build per-core idx16 stream + ids stream
    # sort by (core, group, quarter) to match the segment/`within` layout;
    # ascending src within each segment gives the DMA engines an
    # ascending HBM address walk (bank/row locality) instead of random
    order = np.lexsort((rloc, srcg, qq, grp, ecore))
    e_core = ecore[order]
    e_grp = grp[order]
    e_gpart = gpart[order]
    e_qq = qq[order]
    e_src = srcg[order]

    idx16 = np.full((NCORES, TT * 128), 12500, np.int16)   # pad -> quarter pad row
    idsarr = np.full((NCORES, TT * 128), -1.0, ml_dtypes.bfloat16)

    # stream position for each edge: within (core, g, q) segment
    ckey = (e_core * NGRP + e_grp) * NQ + e_qq
    starts = np.zeros(NCORES * NGRP * NQ + 1, np.int64)
    np.cumsum(cnt.reshape(-1), out=starts[1:])
    within = np.arange(len(order)) - starts[ckey]
    gtile = (tile_base_q[e_qq] + tile_off_gq[e_grp, e_qq]) * 128 + within
    idx16[e_core, gtile] = (e_src - e_qq * QROWS).astype(np.int16)
    idsarr[e_core, gtile] = e_gpart.astype(np.float32)

    # idx16 SBUF layout: stream pos i -> partition i%16 (replicated x8),
    # free slot i//16
    idxw = TT * 128 // 16
    idx_sb = np.zeros((NCORES, 128, idxw), np.int16)
    w16 = idx16.reshape(NCORES, idxw, 16).transpose(0, 2, 1)  # [8,16,idxw]
    idx_sb[:] = np.tile(w16, (1, 8, 1))
    # ids SBUF layout: [128, TT] in GROUP-major tile order (g, then q, t)
    # so each group's SEL builds batch over contiguous columns
    gm_off = np.zeros((NGRP, NQ), np.int64)
    gm_off[:, 1:] = np.cumsum(tiles_gq[:, :-1], axis=1)
    tiles_tot = tiles_gq.sum(axis=1)
    gm_base = np.zeros(NGRP, np.int64)
    gm_base[1:] = np.cumsum(tiles_tot[:-1])
    perm_t = np.zeros(TT, np.int64)
    for g in range(NGRP):
        for q in range(NQ):
            for t in range(tiles_gq[g, q]):
                perm_t[gm_base[g] + gm_off[g, q] + t] = \
                    tile_base_q[q] + tile_off_gq[g, q] + t
    ids_q = idsarr.reshape(NCORES, TT, 128)
    ids_sb = np.ascontiguousarray(
        ids_q[:, perm_t, :].transpose(0, 2, 1))

    # chunk blocks: groups [GBLK*b, GBLK*(b+1)) per quarter
    nblocks = (NGRP + GBLK - 1) // GBLK
    chunks = []   # [block][q] = (idx_slot_start, n_idx, tile_list)
    for b in range(nblocks):
        row = []
        g0, g1 = b * GBLK, min((b + 1) * GBLK, NGRP)
        for q in range(NQ):
            t0 = tile_base_q[q] + tile_off_gq[g0, q]
            ntile = int(tiles_gq[g0:g1, q].sum())
            row.append((int(t0), ntile))
        chunks.append(row)

    dinv_loc = np.zeros((NCORES, 128, NGRP), np.float32)
    binv_row = np.zeros((NCORES, 1, NPAD), np.float32)
    dv = dinv_p.reshape(NCORES, NBLK)
    for k in range(NCORES):
        full = np.zeros(NPAD, np.float32)
        full[:NBLK] = dv[k]
        dinv_loc[k] = full.reshape(NGRP, 128).T
        with np.errstate(divide="ignore"):
            bi = np.where(full > 0, 1.0 / full, 0.0)
        binv_row[k, 0] = bi

    xp = x[perm]
    xblkT = np.zeros((NCORES, C, NPAD), np.float32)
    for k in range(NCORES):
        xblkT[k, :, :NBLK] = xp[k * NBLK:(k + 1) * NBLK].T

    Ws = [np.ascontiguousarray(W, np.float32) for W in (W1, W2, W3)]
    brows = [np.asarray(b, ml_dtypes.bfloat16).reshape(1, -1)
             for b in (b1, b2, b3)]
    has_bias = any(np.any(np.asarray(b) != 0) for b in (b1, b2, b3))
    iota4 = np.tile(np.arange(128, dtype=np.float32)[None, :],
                    (128, 16)).astype(ml_dtypes.bfloat16)

    in_maps = []
    for k in range(NCORES):
        m = {
            "xblkT": np.ascontiguousarray(xblkT[k]),
            "gidx": np.ascontiguousarray(idx_sb[k]),
            "gids": np.ascontiguousarray(ids_sb[k].astype(ml_dtypes.bfloat16)),
            "dinv": np.ascontiguousarray(dinv_loc[k]),
            "iota4": iota4,
            "w1": Ws[0], "w2": Ws[1], "w3": Ws[2],
        }
        if has_bias:
            m["binv"] = np.ascontiguousarray(
                binv_row[k].astype(ml_dtypes.bfloat16))
            m["br1"], m["br2"], m["br3"] = brows
        in_maps.append(m)
    meta = {
        "TT": TT, "idxw": idxw,
        "tiles_gq": tiles_gq.tolist(),
        "tile_base_q": tile_base_q.tolist(),
        "tile_off_gq": tile_off_gq.tolist(),
        "gm_base": gm_base.tolist(),
        "chunks": chunks,
        "has_bias": bool(has_bias),
    }
    return in_maps, meta, perm


def _build(meta):
    from concourse import bacc, bass, mybir, tile
    from concourse.masks import make_identity
    f32 = mybir.dt.float32
    bf16 = mybir.dt.bfloat16
    i16 = mybir.dt.int16

    TT = meta["TT"]
    idxw = meta["idxw"]
    tiles_gq = meta["tiles_gq"]
    tile_base_q = meta["tile_base_q"]
    tile_off_gq = meta["tile_off_gq"]
    gm_base = meta["gm_base"]
    chunks = meta["chunks"]
    nblocks = len(chunks)
    maxtile = max(c[1] for row in chunks for c in [row[q] for q in range(NQ)])

    nc = bacc.Bacc("TRN2", target_bir_lowering=False, debug=False,
                   num_devices=NCORES, num_swdge_queues=4)
    xTd = nc.dram_tensor("xblkT", [C, NPAD], f32, kind="ExternalInput")
    gidx = nc.dram_tensor("gidx", [128, idxw], i16, kind="ExternalInput")
    gids = nc.dram_tensor("gids", [128, TT], bf16, kind="ExternalInput")
    dinv = nc.dram_tensor("dinv", [128, NGRP], f32, kind="ExternalInput")
    has_bias = meta["has_bias"]
    iota_in = nc.dram_tensor("iota4", [128, 16 * 128], bf16,
                             kind="ExternalInput")
    w_in = [nc.dram_tensor(f"w{l+1}", [C, co], f32, kind="ExternalInput")
            for l, co in enumerate([C, C, COUT])]
    if has_bias:
        binv = nc.dram_tensor("binv", [1, NPAD], bf16, kind="ExternalInput")
        br_in = [nc.dram_tensor(f"br{l+1}", [1, co], bf16,
                                kind="ExternalInput")
                 for l, co in enumerate([C, C, COUT])]
    out_d = nc.dram_tensor("out", [NPAD, COUT], f32, kind="ExternalOutput")

    zs = [nc.dram_tensor(f"zs{l}", [NPAD, C], bf16) for l in range(3)]
    zf = [nc.dram_tensor(f"zf{l}", [NCORES * NPAD, C], bf16,
                         addr_space="Shared") for l in range(3)]

    with tile.TileContext(nc) as tc:
        with tc.tile_pool(name="const", bufs=1) as cpool, \
             tc.tile_pool(name="g0", bufs=2) as gp0, \
             tc.tile_pool(name="g1", bufs=2) as gp1, \
             tc.tile_pool(name="g2", bufs=2) as gp2, \
             tc.tile_pool(name="g3", bufs=2) as gp3, \
             tc.tile_pool(name="sel", bufs=4) as selpool, \
             tc.tile_pool(name="work", bufs=4) as wpool, \
             tc.tile_pool(name="ps_g", bufs=2, space="PSUM") as ps_g, \
             tc.tile_pool(name="ps_t", bufs=2, space="PSUM") as ps_t, \
             tc.tile_pool(name="ps_z", bufs=2, space="PSUM") as ps_z:
            gpools = [gp0, gp1, gp2, gp3]

            ident = cpool.tile([128, 128], f32)
            make_identity(nc, ident[:])
            iota4 = cpool.tile([128, 16, 128], bf16)
            nc.sync.dma_start(out=iota4[:], in_=iota_in[:])
            smbuf = cpool.tile([128, NGRP, COUT], f32)
            smx = cpool.tile([128, NGRP], f32)
            sls = cpool.tile([128, NGRP], f32)
            idx_sb = cpool.tile([128, idxw], i16)
            nc.sync.dma_start(out=idx_sb[:], in_=gidx[:])
            ids_sb = cpool.tile([128, TT], bf16)
            nc.sync.dma_start(out=ids_sb[:], in_=gids[:])
            dinv_sb = cpool.tile([128, NGRP], f32)
            nc.sync.dma_start(out=dinv_sb[:], in_=dinv[:])
            w_sb, br_sb = [], []
            for l, co in enumerate([C, C, COUT]):
                w = cpool.tile([128, co], f32, name=f"w_sb{l}")
                nc.sync.dma_start(out=w[:], in_=w_in[l][:])
                w_sb.append(w)
            if has_bias:
                binv_sb = cpool.tile([1, NPAD], bf16)
                nc.sync.dma_start(out=binv_sb[:], in_=binv[:])
                for l, co in enumerate([C, C, COUT]):
                    bt = cpool.tile([1, co], bf16, name=f"br_sb{l}")
                    nc.sync.dma_start(out=bt[:], in_=br_in[l][:])
                    br_sb.append(bt)

            # ---- layer-1 GEMM: zs0 = dinv * (x @ W1), cast bf16 ----
            for g in range(NGRP):
                xt = wpool.tile([128, 128], f32, name="xt")
                nc.sync.dma_start(out=xt[:],
                                  in_=xTd[:, g * 128:(g + 1) * 128])
                psz = ps_z.tile([128, C], f32, name="psz")
                nc.tensor.matmul(out=psz[:], lhsT=xt[:], rhs=w_sb[0][:],
                                 start=True, stop=True)
                zt = wpool.tile([128, C], bf16, name="zt")
                nc.vector.tensor_scalar_mul(out=zt[:], in0=psz[:],
                                            scalar1=dinv_sb[:, g:g + 1])
                nc.sync.dma_start(out=zs[0][g * 128:(g + 1) * 128, :],
                                  in_=zt[:])

            nc.gpsimd.collective_compute(
                "AllGather", mybir.AluOpType.bypass,
                replica_groups=[list(range(NCORES))],
                ins=[zs[0][:, :]], outs=[zf[0][:, :]])

            # ---- per layer: stream-gather aggregation (+ GEMM fusion) ----
            for lay in range(3):
                for b in range(nblocks):
                    bufs = []
                    for q in range(NQ):
                        t0, ntile = chunks[b][q]
                        buf = gpools[q].tile([128, maxtile, C], bf16,
                                             name=f"gb{q}")
                        # <=8 tiles (1024 idxs) per instruction: larger
                        # descriptor batches can exceed the SWDGE ring
                        for s0 in range(0, ntile, 8):
                            ns = min(8, ntile - s0)
                            nc.gpsimd.dma_gather(
                                buf[:, s0:s0 + ns, :],
                                zf[lay][q * QROWS:(q + 1) * QROWS, :],
                                idx_sb[:, (t0 + s0) * 8:(t0 + s0 + ns) * 8],
                                ns * 128, ns * 128, C, queue_num=q)
                        bufs.append((buf, t0))
                    g0 = b * GBLK
                    for g in range(g0, min(g0 + GBLK, NGRP)):
                        # SEL mega-batches over the group's contiguous
                        # group-major ids columns (16 tiles per DVE op)
                        gmb = gm_base[g]
                        ntg = sum(tiles_gq[g])
                        sels = []
                        for s0 in range(0, ntg, 16):
                            w16 = min(16, ntg - s0)
                            selt = selpool.tile([128, 16, 128], bf16,
                                                name="sel")
                            nc.vector.tensor_tensor(
                                out=selt[:, :w16, :],
                                in0=iota4[:, :w16, :],
                                in1=ids_sb[:, gmb + s0:gmb + s0 + w16]
                                    .to_broadcast([128, w16, 128]),
                                op=mybir.AluOpType.is_equal)
                            sels.append(selt)
                        psg = ps_g.tile([128, C], f32, name="psg")
                        jg = 0
                        for q in range(NQ):
                            buf, t0 = bufs[q]
                            tg0 = tile_base_q[q] + tile_off_gq[g][q]
                            nt = tiles_gq[g][q]
                            coff = tg0 - (tile_base_q[q] + tile_off_gq[g0][q])
                            for j in range(nt):
                                nc.tensor.matmul(
                                    out=psg[:],
                                    lhsT=sels[jg // 16][:, jg % 16, :],
                                    rhs=buf[:, coff + j, :],
                                    start=(jg == 0),
                                    stop=(not has_bias and jg == ntg - 1))
                                jg += 1
                        # rank-1 bias: psg += binv_g (x) b_row
                        co = C if lay < 2 else COUT
                        if has_bias:
                            nc.tensor.matmul(
                                out=psg[:, :co],
                                lhsT=binv_sb[:, g * 128:(g + 1) * 128],
                                rhs=br_sb[lay][:], start=False, stop=True)
                        if lay < 2:
                            h = wpool.tile([128, 128], f32, name="h")
                            nc.scalar.activation(
                                out=h[:], in_=psg[:],
                                func=mybir.ActivationFunctionType.Relu,
                                scale=dinv_sb[:, g:g + 1])
                            pst = ps_t.tile([128, 128], f32, name="pst")
                            nc.tensor.transpose(out=pst[:], in_=h[:],
                                                identity=ident[:])
                            ht = wpool.tile([128, 128], f32, name="ht")
                            nc.vector.tensor_copy(out=ht[:], in_=pst[:])
                            co2 = C if lay == 0 else COUT
                            psz = ps_z.tile([128, C], f32, name="psz2")
                            nc.tensor.matmul(out=psz[:, :co2], lhsT=ht[:],
                                             rhs=w_sb[lay + 1][:],
                                             start=True, stop=True)
                            zt = wpool.tile([128, C], bf16, name="zt2")
                            nc.vector.tensor_scalar_mul(
                                out=zt[:, :co2], in0=psz[:, :co2],
                                scalar1=dinv_sb[:, g:g + 1])
                            nc.sync.dma_start(
                                out=zs[lay + 1][g * 128:(g + 1) * 128, :co2],
                                in_=zt[:, :co2])
                        else:
                            nc.vector.tensor_scalar_mul(
                                out=smbuf[:, g, :], in0=psg[:, :COUT],
                                scalar1=dinv_sb[:, g:g + 1])
                if lay < 2:
                    nc.gpsimd.collective_compute(
                        "AllGather", mybir.AluOpType.bypass,
                        replica_groups=[list(range(NCORES))],
                        ins=[zs[lay + 1][:, :]], outs=[zf[lay + 1][:, :]])

            # ---- batched log_softmax over all 98 groups (in-place) ----
            nc.vector.tensor_reduce(
                out=smx[:], in_=smbuf[:], axis=mybir.AxisListType.X,
                op=mybir.AluOpType.max)
            nc.vector.tensor_tensor(
                out=smbuf[:], in0=smbuf[:],
                in1=smx[:].to_broadcast([128, NGRP, COUT]),
                op=mybir.AluOpType.subtract)
            ex = cpool.tile([128, NGRP, COUT], f32, name="exb")
            nc.scalar.activation(
                out=ex[:], in_=smbuf[:],
                func=mybir.ActivationFunctionType.Exp)
            nc.vector.tensor_reduce(
                out=sls[:], in_=ex[:], axis=mybir.AxisListType.X,
                op=mybir.AluOpType.add)
            nc.scalar.activation(
                out=sls[:], in_=sls[:],
                func=mybir.ActivationFunctionType.Ln)
            nc.vector.tensor_tensor(
                out=smbuf[:], in0=smbuf[:],
                in1=sls[:].to_broadcast([128, NGRP, COUT]),
                op=mybir.AluOpType.subtract)
            for g in range(NGRP):
                nc.sync.dma_start(
                    out=out_d[g * 128:(g + 1) * 128, :], in_=smbuf[:, g, :])

    nc.compile()
    return nc


LAST_RES = None


def kernel(x, edge_index, W1, b1, W2, b2, W3, b3):
    import os
    from concourse.bass_utils import run_bass_kernel_spmd

    in_maps, meta, perm = _preprocess(
        x, edge_index, W1, b1, W2, b2, W3, b3)
    nc = _build(meta)
    kw = {}
    if os.environ.get("KERNEL_TRACE", "0") == "1":
        kw["trace"] = True
        if os.environ.get("KERNEL_TMPDIR"):
            kw["tmpdir"] = os.environ["KERNEL_TMPDIR"]
    res = run_bass_kernel_spmd(nc, in_maps, core_ids=list(range(NCORES)), **kw)
    global LAST_RES
    LAST_RES = res
    blocks = [res.results[k]["out"][:NBLK] for k in range(NCORES)]
    outp = np.concatenate(blocks, axis=0)
    out = np.empty((N, COUT), np.float32)
    out[perm] = outp
    return out


# revision 29
# speedup vs baseline: 1.0982x; 1.0982x over previous
"""3-layer GCN forward (GCNConv x3 + log_softmax) on 8 Trainium2 cores.

Strategy (self-contained; shapes hardcoded for N=100000, Cin=Ch=128,
Cout=47, 8 cores): A_hat = D^-1/2 (A+I) D^-1/2 fixed across layers, so
per layer out = dinv_dst * segsum_dst(dinv_src * (H @ W)) + b.

Host: permute nodes into 8 contiguous core blocks (degree-ranked
round-robin so all cores share one loop structure / NEFF). Per core,
edges are sorted into (dst-group g, source-quarter q) segments; each
segment is padded to 128-position tiles. The message gather uses
batched dma_gather instructions (int16 indices rebased per source
quarter of 25088 rows, 4 SWDGE queues in parallel), fetching bf16
feature rows of 256 B from the AllGathered Z replica.

Aggregation per tile of 128 messages: build a selection matrix
SEL[p, r] = (dst_id[p] == r) with one DVE is_equal op (4 tiles per op
via a stride-0 broadcast), then matmul(lhsT=SEL, rhs=messages) into
the group's PSUM accumulator. Bias is a rank-1 matmul
(binv x b, binv = 1/dinv so the later dinv_dst scale restores b).
Post per group: one fused scale+relu, PE transpose, next-layer GEMM,
dinv_src scale, zs write. AllGather (bf16) exchanges Z between layers.
Final layer: scale + log_softmax.
"""
import numpy as np
import ml_dtypes

NCORES = 8
N = 100000
NBLK = 12500
NPAD = 12544            # 98 * 128
NGRP = NPAD // 128      # 98
C = 128
COUT = 47
QROWS = 25088           # rows per source quarter (2 core blocks)
NQ = 4
GBLK = 4                # groups per gather-chunk block


def _preprocess(x, edge_index, W1, b1, W2, b2, W3, b3):
    x = np.asarray(x, np.float32)
    ei = np.asarray(edge_index)
    loop = np.arange(N, dtype=np.int64)
    src = np.concatenate([ei[0], loop]).astype(np.int64)
    dst = np.concatenate([ei[1], loop]).astype(np.int64)

    deg = np.bincount(dst, minlength=N).astype(np.float32)
    dinv = 1.0 / np.sqrt(np.maximum(deg, 1.0))

    rank = np.argsort(-deg, kind="stable")
    perm = np.empty(N, np.int64)
    for k in range(NCORES):
        perm[k * NBLK:(k + 1) * NBLK] = rank[k::NCORES]
    inv = np.empty(N, np.int64)
    inv[perm] = np.arange(N)

    srcp = inv[src]
    dstp = inv[dst]
    ksrc = srcp // NBLK
    srcg = ksrc * NPAD + (srcp - ksrc * NBLK)     # padded-global coords
    dinv_p = dinv[perm]

    ecore = dstp // NBLK
    rloc = dstp - ecore * NBLK
    grp = rloc // 128
    gpart = rloc % 128
    qq = srcg // QROWS

    # counts per (core, group, quarter) -> shared tile structure
    key = (ecore * NGRP + grp) * NQ + qq
    cnt = np.bincount(key, minlength=NCORES * NGRP * NQ) \
        .reshape(NCORES, NGRP, NQ)
    tiles_gq = np.maximum(
        np.ceil(cnt / 128).astype(np.int64).max(axis=0), 1)   # [NGRP, NQ]

    # quarter-major stream layout: for q: for g: tiles_gq[g, q] tiles
    tile_base_q = np.zeros(NQ + 1, np.int64)
    tile_base_q[1:] = np.cumsum(tiles_gq.sum(axis=0))
    # tile index of (g, q): tile_base_q[q] + cumsum over g
    tile_off_gq = np.zeros((NGRP, NQ), np.int64)
    for q in range(NQ):
        tile_off_gq[1:, q] = np.cumsum(tiles_gq[:-1, q])
    TT = int(tile_base_q[-1])          # total tiles per core per layer

    # build per-core idx16 stream + ids stream
    # sort by (core, group, quarter) to match the segment/`within` layout;
    # ascending src within each segment gives the DMA engines an
    # ascending HBM address walk (bank/row locality) instead of random
    order = np.lexsort((rloc, srcg, qq, grp, ecore))
    e_core = ecore[order]
    e_grp = grp[order]
    e_gpart = gpart[order]
    e_qq = qq[order]
    e_src = srcg[order]

    idx16 = np.full((NCORES, TT * 128), 12500, np.int16)   # pad -> quarter pad row
    idsarr = np.full((NCORES, TT * 128), -1.0, ml_dtypes.bfloat16)

    # stream position for each edge: within (core, g, q) segment
    ckey = (e_core * NGRP + e_grp) * NQ + e_qq
    starts = np.zeros(NCORES * NGRP * NQ + 1, np.int64)
    np.cumsum(cnt.reshape(-1), out=starts[1:])
    within = np.arange(len(order)) - starts[ckey]
    gtile = (tile_base_q[e_qq] + tile_off_gq[e_grp, e_qq]) * 128 + within
    idx16[e_core, gtile] = (e_src - e_qq * QROWS).astype(np.int16)
    idsarr[e_core, gtile] = e_gpart.astype(np.float32)

    # idx16 SBUF layout: stream pos i -> partition i%16 (replicated x8),
    # free slot i//16
    idxw = TT * 128 // 16
    idx_sb = np.zeros((NCORES, 128, idxw), np.int16)
    w16 = idx16.reshape(NCORES, idxw, 16).transpose(0, 2, 1)  # [8,16,idxw]
    idx_sb[:] = np.tile(w16, (1, 8, 1))
    # ids SBUF layout: [128, TT] in GROUP-major tile order (g, then q, t)
    # so each group's SEL builds batch over contiguous columns
    gm_off = np.zeros((NGRP, NQ), np.int64)
    gm_off[:, 1:] = np.cumsum(tiles_gq[:, :-1], axis=1)
    tiles_tot = tiles_gq.sum(axis=1)
    gm_base = np.zeros(NGRP, np.int64)
    gm_base[1:] = np.cumsum(tiles_tot[:-1])
    perm_t = np.zeros(TT, np.int64)
    for g in range(NGRP):
        for q in range(NQ):
            for t in range(tiles_gq[g, q]):
                perm_t[gm_base[g] + gm_off[g, q] + t] = \
                    tile_base_q[q] + tile_off_gq[g, q] + t
    ids_q = idsarr.reshape(NCORES, TT, 128)
    ids_sb = np.ascontiguousarray(
        ids_q[:, perm_t, :].transpose(0, 2, 1))

    # chunk blocks: groups [GBLK*b, GBLK*(b+1)) per quarter
    nblocks = (NGRP + GBLK - 1) // GBLK
    chunks = []   # [block][q] = (idx_slot_start, n_idx, tile_list)
    for b in range(nblocks):
        row = []
        g0, g1 = b * GBLK, min((b + 1) * GBLK, NGRP)
        for q in range(NQ):
            t0 = tile_base_q[q] + tile_off_gq[g0, q]
            ntile = int(tiles_gq[g0:g1, q].sum())
            row.append((int(t0), ntile))
        chunks.append(row)

    dinv_loc = np.zeros((NCORES, 128, NGRP), np.float32)
    binv_row = np.zeros((NCORES, 1, NPAD), np.float32)
    dv = dinv_p.reshape(NCORES, NBLK)
    for k in range(NCORES):
        full = np.zeros(NPAD, np.float32)
        full[:NBLK] = dv[k]
        dinv_loc[k] = full.reshape(NGRP, 128).T
        with np.errstate(divide="ignore"):
            bi = np.where(full > 0, 1.0 / full, 0.0)
        binv_row[k, 0] = bi

    xp = x[perm]
    xblkT = np.zeros((NCORES, C, NPAD), np.float32)
    for k in range(NCORES):
        xblkT[k, :, :NBLK] = xp[k * NBLK:(k + 1) * NBLK].T

    Ws = [np.ascontiguousarray(W, np.float32) for W in (W1, W2, W3)]
    brows = [np.asarray(b, ml_dtypes.bfloat16).reshape(1, -1)
             for b in (b1, b2, b3)]
    has_bias = any(np.any(np.asarray(b) != 0) for b in (b1, b2, b3))
    iota4 = np.tile(np.arange(128, dtype=np.float32)[None, :],
                    (128, 16)).astype(ml_dtypes.bfloat16)

    in_maps = []
    for k in range(NCORES):
        m = {
            "xblkT": np.ascontiguousarray(xblkT[k]),
            "gidx": np.ascontiguousarray(idx_sb[k]),
            "gids": np.ascontiguousarray(ids_sb[k].astype(ml_dtypes.bfloat16)),
            "dinv": np.ascontiguousarray(dinv_loc[k]),
            "iota4": iota4,
            "w1": Ws[0], "w2": Ws[1], "w3": Ws[2],
        }
        if has_bias:
            m["binv"] = np.ascontiguousarray(
                binv_row[k].astype(ml_dtypes.bfloat16))
            m["br1"], m["br2"], m["br3"] = brows
        in_maps.append(m)
    meta = {
        "TT": TT, "idxw": idxw,
        "tiles_gq": tiles_gq.tolist(),
        "tile_base_q": tile_base_q.tolist(),
        "tile_off_gq": tile_off_gq.tolist(),
        "gm_base": gm_base.tolist(),
        "chunks": chunks,
        "has_bias": bool(has_bias),
    }
    return in_maps, meta, perm


def _build(meta):
    from concourse import bacc, bass, mybir, tile
    from concourse.masks import make_identity
    f32 = mybir.dt.float32
    bf16 = mybir.dt.bfloat16
    i16 = mybir.dt.int16

    TT = meta["TT"]
    idxw = meta["idxw"]
    tiles_gq = meta["tiles_gq"]
    tile_base_q = meta["tile_base_q"]
    tile_off_gq = meta["tile_off_gq"]
    gm_base = meta["gm_base"]
    chunks = meta["chunks"]
    nblocks = len(chunks)
    maxtile = max(c[1] for row in chunks for c in [row[q] for q in range(NQ)])

    nc = bacc.Bacc("TRN2", target_bir_lowering=False, debug=False,
                   num_devices=NCORES, num_swdge_queues=4)
    xTd = nc.dram_tensor("xblkT", [C, NPAD], f32, kind="ExternalInput")
    gidx = nc.dram_tensor("gidx", [128, idxw], i16, kind="ExternalInput")
    gids = nc.dram_tensor("gids", [128, TT], bf16, kind="ExternalInput")
    dinv = nc.dram_tensor("dinv", [128, NGRP], f32, kind="ExternalInput")
    has_bias = meta["has_bias"]
    iota_in = nc.dram_tensor("iota4", [128, 16 * 128], bf16,
                             kind="ExternalInput")
    w_in = [nc.dram_tensor(f"w{l+1}", [C, co], f32, kind="ExternalInput")
            for l, co in enumerate([C, C, COUT])]
    if has_bias:
        binv = nc.dram_tensor("binv", [1, NPAD], bf16, kind="ExternalInput")
        br_in = [nc.dram_tensor(f"br{l+1}", [1, co], bf16,
                                kind="ExternalInput")
                 for l, co in enumerate([C, C, COUT])]
    out_d = nc.dram_tensor("out", [NPAD, COUT], f32, kind="ExternalOutput")

    zs = [nc.dram_tensor(f"zs{l}", [NPAD, C], bf16) for l in range(3)]
    zf = [nc.dram_tensor(f"zf{l}", [NCORES * NPAD, C], bf16,
                         addr_space="Shared") for l in range(3)]

    with tile.TileContext(nc) as tc:
        with tc.tile_pool(name="const", bufs=1) as cpool, \
             tc.tile_pool(name="g0", bufs=2) as gp0, \
             tc.tile_pool(name="g1", bufs=2) as gp1, \
             tc.tile_pool(name="g2", bufs=2) as gp2, \
             tc.tile_pool(name="g3", bufs=2) as gp3, \
             tc.tile_pool(name="sel", bufs=4) as selpool, \
             tc.tile_pool(name="work", bufs=4) as wpool, \
             tc.tile_pool(name="ps_g", bufs=2, space="PSUM") as ps_g, \
             tc.tile_pool(name="ps_t", bufs=2, space="PSUM") as ps_t, \
             tc.tile_pool(name="ps_z", bufs=2, space="PSUM") as ps_z:
            gpools = [gp0, gp1, gp2, gp3]

            ident = cpool.tile([128, 128], f32)
            make_identity(nc, ident[:])
            iota4 = cpool.tile([128, 16, 128], bf16)
            nc.sync.dma_start(out=iota4[:], in_=iota_in[:])
            smbuf = cpool.tile([128, NGRP, COUT], f32)
            smx = cpool.tile([128, NGRP], f32)
            sls = cpool.tile([128, NGRP], f32)
            idx_sb = cpool.tile([128, idxw], i16)
            nc.sync.dma_start(out=idx_sb[:], in_=gidx[:])
            ids_sb = cpool.tile([128, TT], bf16)
            nc.sync.dma_start(out=ids_sb[:], in_=gids[:])
            dinv_sb = cpool.tile([128, NGRP], f32)
            nc.sync.dma_start(out=dinv_sb[:], in_=dinv[:])
            w_sb, br_sb = [], []
            for l, co in enumerate([C, C, COUT]):
                w = cpool.tile([128, co], f32, name=f"w_sb{l}")
                nc.sync.dma_start(out=w[:], in_=w_in[l][:])
                w_sb.append(w)
            if has_bias:
                binv_sb = cpool.tile([1, NPAD], bf16)
                nc.sync.dma_start(out=binv_sb[:], in_=binv[:])
                for l, co in enumerate([C, C, COUT]):
                    bt = cpool.tile([1, co], bf16, name=f"br_sb{l}")
                    nc.sync.dma_start(out=bt[:], in_=br_in[l][:])
                    br_sb.append(bt)

            # ---- layer-1 GEMM: zs0 = dinv * (x @ W1), cast bf16 ----
            for g in range(NGRP):
                xt = wpool.tile([128, 128], f32, name="xt")
                nc.sync.dma_start(out=xt[:],
                                  in_=xTd[:, g * 128:(g + 1) * 128])
                psz = ps_z.tile([128, C], f32, name="psz")
                nc.tensor.matmul(out=psz[:], lhsT=xt[:], rhs=w_sb[0][:],
                                 start=True, stop=True)
                zt = wpool.tile([128, C], bf16, name="zt")
                nc.vector.tensor_scalar_mul(out=zt[:], in0=psz[:],
                                            scalar1=dinv_sb[:, g:g + 1])
                nc.sync.dma_start(out=zs[0][g * 128:(g + 1) * 128, :],
                                  in_=zt[:])

            nc.gpsimd.collective_compute(
                "AllGather", mybir.AluOpType.bypass,
                replica_groups=[list(range(NCORES))],
                ins=[zs[0][:, :]], outs=[zf[0][:, :]])

            # ---- per layer: stream-gather aggregation (+ GEMM fusion) ----
            for lay in range(3):
                for b in range(nblocks):
                    bufs = []
                    for q in range(NQ):
                        t0, ntile = chunks[b][q]
                        buf = gpools[q].tile([128, maxtile, C], bf16,
                                             name=f"gb{q}")
                        # <=16 tiles (2048 idxs = 129 descs/engine) per
                        # instruction: stays under the ~256-desc/engine
                        # SWDGE ring (4224-idx instructions deadlocked)
                        for s0 in range(0, ntile, 16):
                            ns = min(16, ntile - s0)
                            nc.gpsimd.dma_gather(
                                buf[:, s0:s0 + ns, :],
                                zf[lay][q * QROWS:(q + 1) * QROWS, :],
                                idx_sb[:, (t0 + s0) * 8:(t0 + s0 + ns) * 8],
                                ns * 128, ns * 128, C, queue_num=q,
                                single_packet=False)
                        bufs.append((buf, t0))
                    g0 = b * GBLK
                    for g in range(g0, min(g0 + GBLK, NGRP)):
                        # SEL mega-batches over the group's contiguous
                        # group-major ids columns (16 tiles per DVE op)
                        gmb = gm_base[g]
                        ntg = sum(tiles_gq[g])
                        sels = []
                        for s0 in range(0, ntg, 16):
                            w16 = min(16, ntg - s0)
                            selt = selpool.tile([128, 16, 128], bf16,
                                                name="sel")
                            nc.vector.tensor_tensor(
                                out=selt[:, :w16, :],
                                in0=iota4[:, :w16, :],
                                in1=ids_sb[:, gmb + s0:gmb + s0 + w16]
                                    .to_broadcast([128, w16, 128]),
                                op=mybir.AluOpType.is_equal)
                            sels.append(selt)
                        psg = ps_g.tile([128, C], f32, name="psg")
                        jg = 0
                        for q in range(NQ):
                            buf, t0 = bufs[q]
                            tg0 = tile_base_q[q] + tile_off_gq[g][q]
                            nt = tiles_gq[g][q]
                            coff = tg0 - (tile_base_q[q] + tile_off_gq[g0][q])
                            for j in range(nt):
                                nc.tensor.matmul(
                                    out=psg[:],
                                    lhsT=sels[jg // 16][:, jg % 16, :],
                                    rhs=buf[:, coff + j, :],
                                    start=(jg == 0),
                                    stop=(not has_bias and jg == ntg - 1))
                                jg += 1
                        # rank-1 bias: psg += binv_g (x) b_row
                        co = C if lay < 2 else COUT
                        if has_bias:
                            nc.tensor.matmul(
                                out=psg[:, :co],
                                lhsT=binv_sb[:, g * 128:(g + 1) * 128],
                                rhs=br_sb[lay][:], start=False, stop=True)
                        if lay < 2:
                            h = wpool.tile([128, 128], f32, name="h")
                            nc.scalar.activation(
                                out=h[:], in_=psg[:],
                                func=mybir.ActivationFunctionType.Relu,
                                scale=dinv_sb[:, g:g + 1])
                            pst = ps_t.tile([128, 128], f32, name="pst")
                            nc.tensor.transpose(out=pst[:], in_=h[:],
                                                identity=ident[:])
                            ht = wpool.tile([128, 128], f32, name="ht")
                            nc.vector.tensor_copy(out=ht[:], in_=pst[:])
                            co2 = C if lay == 0 else COUT
                            psz = ps_z.tile([128, C], f32, name="psz2")
                            nc.tensor.matmul(out=psz[:, :co2], lhsT=ht[:],
                                             rhs=w_sb[lay + 1][:],
                                             start=True, stop=True)
                            zt = wpool.tile([128, C], bf16, name="zt2")
                            nc.vector.tensor_scalar_mul(
                                out=zt[:, :co2], in0=psz[:, :co2],
                                scalar1=dinv_sb[:, g:g + 1])
                            nc.sync.dma_start(
                                out=zs[lay + 1][g * 128:(g + 1) * 128, :co2],
                                in_=zt[:, :co2])
                        else:
                            nc.vector.tensor_scalar_mul(
                                out=smbuf[:, g, :], in0=psg[:, :COUT],
                                scalar1=dinv_sb[:, g:g + 1])
                if lay < 2:
                    nc.gpsimd.collective_compute(
                        "AllGather", mybir.AluOpType.bypass,
                        replica_groups=[list(range(NCORES))],
                        ins=[zs[lay + 1][:, :]], outs=[zf[lay + 1][:, :]])

            # ---- batched log_softmax over all 98 groups (in-place) ----
            nc.vector.tensor_reduce(
                out=smx[:], in_=smbuf[:], axis=mybir.AxisListType.X,
                op=mybir.AluOpType.max)
            nc.vector.tensor_tensor(
                out=smbuf[:], in0=smbuf[:],
                in1=smx[:].to_broadcast([128, NGRP, COUT]),
                op=mybir.AluOpType.subtract)
            ex = cpool.tile([128, NGRP, COUT], f32, name="exb")
            nc.scalar.activation(
                out=ex[:], in_=smbuf[:],
                func=mybir.ActivationFunctionType.Exp)
            nc.vector.tensor_reduce(
                out=sls[:], in_=ex[:], axis=mybir.AxisListType.X,
                op=mybir.AluOpType.add)
            nc.scalar.activation(
                out=sls[:], in_=sls[:],
                func=mybir.ActivationFunctionType.Ln)
            nc.vector.tensor_tensor(
                out=smbuf[:], in0=smbuf[:],
                in1=sls[:].to_broadcast([128, NGRP, COUT]),
                op=mybir.AluOpType.subtract)
            for g in range(NGRP):
                nc.sync.dma_start(
                    out=out_d[g * 128:(g + 1) * 128, :], in_=smbuf[:, g, :])

    nc.compile()
    return nc


LAST_RES = None


def kernel(x, edge_index, W1, b1, W2, b2, W3, b3):
    import os
    from concourse.bass_utils import run_bass_kernel_spmd

    in_maps, meta, perm = _preprocess(
        x, edge_index, W1, b1, W2, b2, W3, b3)
    nc = _build(meta)
    kw = {}
    if os.environ.get("KERNEL_TRACE", "0") == "1":
        kw["trace"] = True
        if os.environ.get("KERNEL_TMPDIR"):
            kw["tmpdir"] = os.environ["KERNEL_TMPDIR"]
    res = run_bass_kernel_spmd(nc, in_maps, core_ids=list(range(NCORES)), **kw)
    global LAST_RES
    LAST_RES = res
    blocks = [res.results[k]["out"][:NBLK] for k in range(NCORES)]
    outp = np.concatenate(blocks, axis=0)
    out = np.empty((N, COUT), np.float32)
    out[perm] = outp
    return out
